# revision 53
# baseline (speedup 1.0000x reference)
"""Pointer-network decode kernel for 8 Trainium2 NeuronCores (v2).

Data-parallel over batch: B=64 -> 8 batches per core. Each core runs the
full T-step attention/GRU decode for its batch slice; the host shards
inputs (with layout pre-transposes/casts) and concatenates outputs.

Per core (B_loc=8, N=2048, D=H=P=256):
  t=0: proj = W_seq^T seqT (PE, bf16, psum groups [128, 1024]); tanh w/
       hW-bias (ACT, psum -> bf16 th); proj cache psum -> bf16 (DVE);
       score matmuls v8 x th accumulate into psc[8, 2048] on top of a
       mask-bias init pass (identity-stationary matmul).  Emission is
       software-pipelined: scores for group g-1 issue behind proj g.
  t>0: tanh from bf16 proj cache, same lagged score matmuls.
  softmax: exp straight from psum (no max subtraction; scores are O(1)),
       unnormalized bf16 w rows + fp32 row-sum accum; host divides.
  vec: transpose w rows (PE) -> wT bf16; vec_b = sum_c wT-col x seqO
       chunk matmuls; normalized against the exp row-sum via a
       reciprocal row broadcast (gpsimd) folded into the psum evac.
  GRU in transposed space: PE bf16 matmuls, psum evacuated fused with
       bias add (DVE scalar_tensor_tensor), fp32 gate math.
All PE stationaries are bf16 (fp32 stationaries cost a double LDWEIGHTS
pass).  Host normalizes wout by esum and upcasts to fp32.
"""

import numpy as np
import ml_dtypes

B, N, D, H, P = 64, 2048, 256, 256, 256
NCORES = 8
BL = B // NCORES          # batch per core
NEG_INF = -1e9

_CACHE = {}


def _build(T: int, split: bool = True, masked: bool = False):
    """Build the Bass program (one core's SPMD program) for T decode steps.

    masked=False specializes for an all-ones sequence_mask (the mask bias
    rows are all zero, so the psc init pass is dropped and the first score
    matmul starts the accumulation)."""
    from contextlib import ExitStack
    import concourse.bass as bass
    import concourse.tile as tile
    from concourse import mybir, library_config

    f32 = mybir.dt.float32
    bf16 = mybir.dt.bfloat16
    f8 = mybir.dt.float8e4
    AF = mybir.ActivationFunctionType
    ALU = mybir.AluOpType
    DR = mybir.MatmulPerfMode.DoubleRow

    nc = bass.Bass()

    # ---- DRAM I/O (per-core shapes) ----
    d_seqT = nc.dram_tensor("seqT", [128, BL, 2, N], bf16, kind="ExternalInput")
    d_seqO = nc.dram_tensor("seqO", [128, BL, N // 256, 2, D], f8,
                            kind="ExternalInput")
    d_wseq = nc.dram_tensor("wseq", [128, 2, P], bf16, kind="ExternalInput")
    d_v8 = nc.dram_tensor("v8", [128, 2, BL, BL], bf16, kind="ExternalInput")
    if masked:
        d_mb8 = nc.dram_tensor("mb8", [BL, N], bf16, kind="ExternalInput")
    d_id8 = nc.dram_tensor("id8", [BL, BL], bf16, kind="ExternalInput")
    d_id8f = nc.dram_tensor("id8f", [BL, BL], f32, kind="ExternalInput")
    d_h0T = nc.dram_tensor("h0T", [128, 2, BL], f32, kind="ExternalInput")
    d_wh = nc.dram_tensor("wh", [128, 2, P], bf16, kind="ExternalInput")
    d_wihT = nc.dram_tensor("wihT", [128, 2, 3 * H], bf16, kind="ExternalInput")
    d_whhT = nc.dram_tensor("whhT", [128, 2, 3 * H], bf16, kind="ExternalInput")
    d_bih8 = nc.dram_tensor("bih8", [128, 6, BL], f32, kind="ExternalInput")
    d_bhh8 = nc.dram_tensor("bhh8", [128, 6, BL], f32, kind="ExternalInput")
    d_wout = nc.dram_tensor("wout", [BL, T, N], bf16, kind="ExternalOutput")
    d_esum = nc.dram_tensor("esum", [T, BL], f32, kind="ExternalOutput")

    NC4 = N // 512    # score psum banks
    NCH = N // 128    # seqO chunks

    with tile.TileContext(nc) as tc, ExitStack() as ctx:
        # PSUM (16KB/partition): psc [8, 2048] f32 8K + pg 2 x 4K.  All
        # small matmul outputs borrow pg slots outside the proj pipeline.
        pscp = ctx.enter_context(tc.tile_pool(name="psc", bufs=1, space="PSUM"))
        pgp = ctx.enter_context(tc.tile_pool(name="pg", bufs=2, space="PSUM"))

        cpool = ctx.enter_context(tc.tile_pool(name="consts", bufs=1))
        blkp = ctx.enter_context(tc.tile_pool(name="blk", bufs=13))
        sqop = ctx.enter_context(tc.tile_pool(name="sqo", bufs=8))
        thp = ctx.enter_context(tc.tile_pool(name="th", bufs=6))
        rowp = ctx.enter_context(tc.tile_pool(name="rows", bufs=2))
        hp = ctx.enter_context(tc.tile_pool(name="hstate", bufs=2))

        # ---- constants / weights into SBUF (step-0-critical first) ----
        # wseq split by kh so the first proj matmul (kh=0) unblocks early
        wseq_sb = cpool.tile([128, 2, P], bf16, tag="wseq")
        nc.sync.dma_start(wseq_sb[:, 0, :], d_wseq[:, 0, :])
        nc.sync.dma_start(wseq_sb[:, 1, :], d_wseq[:, 1, :])
        hT0 = hp.tile([128, 2, BL], f32, tag="hT")
        nc.sync.dma_start(hT0[:], d_h0T[:])
        wh_sb = cpool.tile([128, 2, P], bf16, tag="wh")
        nc.sync.dma_start(wh_sb[:], d_wh[:])
        ones_f = cpool.tile([1, 128], f32, tag="ones_f")
        nc.gpsimd.memset(ones_f[:], 1.0)

        # ---- sequence loads: front batches split finer so b0 lands fast;
        # n-major piece order so the first proj group waits on 4 pieces ----
        seqT_b = [blkp.tile([128, 2, N], bf16, tag="blk", name=f"seqT{b}")
                  for b in range(BL)]

        def seq_pieces(b):
            tl = seqT_b[b]
            nsplit = 4 if b < 2 else 2
            for q in range(nsplit):
                qs = slice(q * (N // nsplit), (q + 1) * (N // nsplit))
                for dh in range(2):
                    nc.sync.dma_start(tl[:, dh, qs], d_seqT[:, b, dh, qs])

        seq_pieces(0)
        # scores (v8) and the transposes (id8) are not needed until a few
        # microseconds in; b0's pieces go out first
        v8_sb = cpool.tile([128, 2, BL, BL], bf16, tag="v8")
        nc.sync.dma_start(v8_sb[:], d_v8[:])
        if masked:
            mb8_sb = cpool.tile([BL, N], bf16, tag="mb8")
            nc.sync.dma_start(mb8_sb[:], d_mb8[:])
        seq_pieces(1)
        id8_sb = cpool.tile([BL, BL], bf16, tag="id8")
        nc.sync.dma_start(id8_sb[:], d_id8[:])
        id8f_sb = cpool.tile([BL, BL], f32, tag="id8f")
        nc.sync.dma_start(id8f_sb[:], d_id8f[:])
        for b in range(2, BL):
            seq_pieces(b)

        wihT_sb = cpool.tile([128, 2, 3 * H], bf16, tag="wihT")
        nc.sync.dma_start(wihT_sb[:], d_wihT[:])
        whhT_sb = cpool.tile([128, 2, 3 * H], bf16, tag="whhT")
        nc.sync.dma_start(whhT_sb[:], d_whhT[:])
        bih8_sb = cpool.tile([128, 6, BL], f32, tag="bih8")
        nc.sync.dma_start(bih8_sb[:], d_bih8[:])
        bhh8_sb = cpool.tile([128, 6, BL], f32, tag="bhh8")
        nc.sync.dma_start(bhh8_sb[:], d_bhh8[:])

        seqO_b = []
        for b in range(BL):
            tl = sqop.tile([128, NCH // 2, 2, D], f8, tag="sqo",
                           name=f"seqO{b}")
            nc.sync.dma_start(tl[:], d_seqO[:, b, :, :, :])
            seqO_b.append(tl)

        projc_b = [None] * BL
        hT_sb = hT0
        hTb_sb = hp.tile([128, 2, BL], bf16, tag="hTb")
        nc.vector.tensor_copy(hTb_sb[:], hT_sb[:])

        def compute_hW(hW_sb, hTb, bs, bw, tag):
            """hW^T = W_h^T h for batches [bs, bs+bw) -> hW_sb[:, :, bs:]."""
            p_hw = pgp.tile([128, 2, bw], f32, tag="pg", name=f"p_hw{tag}")
            for mh in range(2):
                for kh in range(2):
                    nc.tensor.matmul(
                        p_hw[:, mh, :],
                        wh_sb[:, kh, mh * 128:(mh + 1) * 128],
                        hTb[:, kh, bs:bs + bw],
                        start=(kh == 0), stop=(kh == 1),
                    )
            nc.scalar.copy(hW_sb[:, :, bs:bs + bw], p_hw[:])

        hW_sb = hp.tile([128, 2, BL], f32, tag="hW")
        compute_hW(hW_sb, hTb_sb, 0, BL, "0")

        for t in range(T):
            # ---- score psum (mask-bias init only when the mask is real) ----
            psc = pscp.tile([BL, N], f32, tag="sc", name=f"psc{t}")
            started = set()
            if masked:
                for c in range(NC4):
                    cs = slice(c * 512, (c + 1) * 512)
                    nc.tensor.matmul(
                        psc[:, cs], id8_sb[:], mb8_sb[:, cs],
                        start=True, stop=False, skip_group_check=True,
                    )
                    started.add(c)

            # ---- proj(+cache)/tanh, with score matmuls lagged one group ----
            def emit_score(g):
                # each psc bank's final accumulation lands in a (b7, mh1)
                # group (banks are written by disjoint ch groups at t=0)
                b, mh, th_t, ns = g
                for c2 in range(len(ns)):
                    c = ns[c2] // 512
                    nc.tensor.matmul(
                        psc[:, ns[c2]:ns[c2] + 512],
                        v8_sb[:, mh, b, :],
                        th_t[:, c2 * 512:(c2 + 1) * 512],
                        start=(c not in started),
                        stop=(b == BL - 1 and mh == 1),
                        skip_group_check=True,
                    )
                    started.add(c)

            pend = []
            for b in range(BL):
                for mh in range(2):
                    if t == 0:
                        if projc_b[b] is None:
                            projc_b[b] = blkp.tile(
                                [128, 2, N], bf16, tag="blk", name=f"projc{b}")
                        for ch in range(2):
                            cs = slice(ch * 1024, (ch + 1) * 1024)
                            pg = pgp.tile([128, 1024], f32, tag="pg",
                                          name=f"pg{b}_{mh}_{ch}")
                            for kh in range(2):
                                for c2 in range(2):
                                    ns = slice(ch * 1024 + c2 * 512,
                                               ch * 1024 + (c2 + 1) * 512)
                                    nc.tensor.matmul(
                                        pg[:, c2 * 512:(c2 + 1) * 512],
                                        wseq_sb[:, kh, mh * 128:(mh + 1) * 128],
                                        seqT_b[b][:, kh, ns],
                                        start=(kh == 0), stop=(kh == 1),
                                    )
                                # score matmuls (lagged 2 groups) slot
                                # between the kh passes, widening the psum
                                # accumulation RAW distance
                                if kh == 0 and len(pend) >= 2:
                                    emit_score(pend.pop(0))
                            th = thp.tile([128, 1024], bf16, tag="th")
                            nc.scalar.activation(
                                th[:], pg[:], AF.Tanh,
                                bias=hW_sb[:, mh, b:b + 1],
                            )
                            if T > 1:
                                nc.vector.tensor_copy(projc_b[b][:, mh, cs],
                                                      pg[:])
                            pend.append((b, mh, th,
                                         [ch * 1024, ch * 1024 + 512]))
                    else:
                        if len(pend) >= 2:
                            emit_score(pend.pop(0))
                        th1 = thp.tile([128, N], bf16, tag="th")
                        nc.scalar.activation(
                            th1[:], projc_b[b][:, mh, :], AF.Tanh,
                            bias=hW_sb[:, mh, b:b + 1],
                        )
                        pend.append((b, mh, th1, [0, 512, 1024, 1536]))
            while pend:
                emit_score(pend.pop(0))

            HB = BL // 2

            def gates_T(src_bf, wT_w, bias8, bs, name):
                pgt = pgp.tile([128, 6, HB], f32, tag="pg", name=f"pgt{name}")
                for gh in range(6):
                    for dh in range(2):
                        nc.tensor.matmul(
                            pgt[:, gh, :],
                            wT_w[:, dh, gh * 128:(gh + 1) * 128],
                            src_bf[:, dh, :],
                            start=(dh == 0), stop=(dh == 1),
                        )
                g_sb = hp.tile([128, 6, HB], f32, tag=f"g{name[-1]}")
                nc.vector.scalar_tensor_tensor(
                    g_sb[:], pgt[:], 1.0, bias8[:, :, bs],
                    op0=ALU.bypass, op1=ALU.add,
                )
                return g_sb

            # h-gates don't depend on this step's attention: emit them
            # before exp so the PE computes them during the softmax
            gh_halves = []
            if t < T - 1:
                for half in range(2):
                    bs = slice(half * HB, half * HB + HB)
                    gh_halves.append(gates_T(
                        hTb_sb[:, :, bs], whhT_sb, bhh8_sb, bs,
                        f"gh{t}_{half}"))

            # ---- softmax numerator: exp straight from psum, bf16 out ----
            w_eb = rowp.tile([BL, N], bf16, tag="w_eb")
            esum = rowp.tile([BL, 1], f32, tag="esum")
            nc.scalar.activation(
                w_eb[:], psc[:], AF.Exp, accum_out=esum[:],
            )
            # two parallel queues halve the output-row DMA on the exit path
            nc.sync.dma_start(d_wout[0:BL // 2, t, :], w_eb[0:BL // 2, :])
            nc.sync.dma_start(d_wout[BL // 2:, t, :], w_eb[BL // 2:, :])
            nc.sync.dma_start(d_esum[t, :], esum[:, 0])

            if t == T - 1:
                break   # final step: w/esum are out, the GRU state is dead

            # ---- wT: transpose w rows into [n-part, b] fp8 chunks ----
            # (emitted first: they only need w_eb, so the PE is not queued
            # behind the esum-transpose/reciprocal chain)
            wT_sb = rowp.tile([128, NCH, 2 * BL], f8, tag="wT")
            nc.gpsimd.memset(wT_sb[:, :, 1:2 * BL:2], 0.0)
            p_wT = pgp.tile([128, NCH, BL], bf16, tag="pg", name=f"p_wT{t}")
            for c in range(NCH):
                nc.tensor.transpose(
                    p_wT[:, c, :], w_eb[:, c * 128:(c + 1) * 128], id8_sb[:],
                )
            # ---- 1/esum as a broadcast row set [128, 8] fp32 ----
            p_esT = pgp.tile([1, BL], f32, tag="pg", name=f"p_esT{t}")
            nc.tensor.transpose(p_esT[:], esum[:], id8f_sb[:])
            rrow = rowp.tile([1, BL], f32, tag="rrow")
            nc.vector.reciprocal(rrow[:], p_esT[:])
            p_rb = pgp.tile([128, BL], f32, tag="pg", name=f"p_rb{t}")
            nc.tensor.matmul(p_rb[:], ones_f[:], rrow[:], start=True, stop=True)
            rb = rowp.tile([128, BL], f32, tag="rb")
            nc.vector.tensor_copy(rb[:], p_rb[:])
            # dual-fp8 ldweights needs a 16-bit-aligned base and k-stride
            # and >1 column: batch b's weights live at even column 2b with
            # a zero pad at 2b+1 (its psum row is never read).
            nc.vector.tensor_copy(wT_sb[:, :, 0:2 * BL:2], p_wT[:])

            # ---- vec + GRU + next hW, in batch halves: the second half's
            # work overlaps the next step's tanh for the first half ----
            p_vT = pscp.tile([128, 2, BL], f32, tag="sc", name=f"p_vT{t}")
            hT_new = hp.tile([128, 2, BL], f32, tag="hT", name=f"hT{t}")
            hTb_new = hp.tile([128, 2, BL], bf16, tag="hTb", name=f"hTb{t}")
            hW_new = hp.tile([128, 2, BL], f32, tag="hW", name=f"hW{t}")
            HB = BL // 2
            for half in range(2):
                bs = slice(half * HB, half * HB + HB)
                # vec rows via fp8 DoubleRow (256-wide contraction/pass; vec
                # feeds only the GRU whose output sensitivity is ~2%, so
                # fp8 noise is invisible)
                for b in range(half * HB, half * HB + HB):
                    p_vr = pgp.tile([2, D], f32, tag="pg",
                                    name=f"p_vr{t}_{b}")
                    for c in range(NCH // 2):
                        nc.tensor.matmul(
                            p_vr[:], wT_sb[:, 2 * c:2 * c + 2,
                                           2 * b:2 * b + 2],
                            seqO_b[b][:, c, :, :],
                            start=(c == 0), stop=(c == NCH // 2 - 1),
                            skip_group_check=True, perf_mode=DR,
                        )
                    vrow = rowp.tile([1, D], f32, tag="vrow")
                    nc.scalar.copy(vrow[:], p_vr[0:1, :])
                    for dh in range(2):
                        nc.tensor.transpose(
                            p_vT[:, dh, b:b + 1],
                            vrow[0:1, dh * 128:(dh + 1) * 128],
                            id8f_sb[:1, :1],
                        )
                # normalize + cast: vecTb = (vec_e^T) * (1/esum) in bf16
                vecTb = hp.tile([128, 2, HB], bf16, tag="vecTb")
                for dh in range(2):
                    nc.vector.tensor_tensor(
                        vecTb[:, dh, :], p_vT[:, dh, bs], rb[:, bs],
                        op=ALU.mult)

                gx = gates_T(vecTb, wihT_sb, bih8_sb, bs, f"gx{t}_{half}")
                gh_ = gh_halves[half]

                # r,z = sigmoid via 0.5*tanh(0.5x)+0.5
                rz_in = hp.tile([128, 4, HB], f32, tag="rz_in")
                nc.vector.tensor_tensor(rz_in[:], gx[:, 0:4, :],
                                        gh_[:, 0:4, :], op=ALU.add)
                rz = hp.tile([128, 4, HB], f32, tag="rz")
                nc.scalar.activation(rz[:], rz_in[:], AF.Tanh, scale=0.5)
                nc.vector.tensor_scalar(
                    rz[:], rz[:], 0.5, 0.5, op0=ALU.mult, op1=ALU.add,
                )
                # n = tanh(xn + r*hn)
                n_in = hp.tile([128, 2, HB], f32, tag="n_in")
                nc.vector.tensor_tensor(n_in[:], rz[:, 0:2, :],
                                        gh_[:, 4:6, :], op=ALU.mult)
                nc.vector.tensor_tensor(n_in[:], n_in[:], gx[:, 4:6, :],
                                        op=ALU.add)
                n_t = hp.tile([128, 2, HB], f32, tag="n_t")
                nc.scalar.activation(n_t[:], n_in[:], AF.Tanh)
                # h' = n + z*(h - n)
                hmn = hp.tile([128, 2, HB], f32, tag="hmn")
                nc.vector.tensor_tensor(hmn[:], hT_sb[:, :, bs], n_t[:],
                                        op=ALU.subtract)
                nc.vector.tensor_tensor(hmn[:], rz[:, 2:4, :], hmn[:],
                                        op=ALU.mult)
                nc.vector.tensor_tensor(hT_new[:, :, bs], n_t[:], hmn[:],
                                        op=ALU.add)
                nc.vector.tensor_copy(hTb_new[:, :, bs], hT_new[:, :, bs])
                compute_hW(hW_new, hTb_new, half * HB, HB, f"{t}_{half}")
            hT_sb = hT_new
            hTb_sb = hTb_new
            hW_sb = hW_new

    if split:
        _split_multiwaits(nc, mybir)
    return nc


def _split_multiwaits(nc, mybir):
    """Walrus gives each lowered TPB instruction a single sem-wait slot;
    Tile happily emits several. Peel surplus waits onto same-engine NoOps
    inserted right before the instruction (semantically identical: the
    engine stalls at the same program point)."""
    skip = ("InstNoOp", "InstEventSemaphore")
    for f in nc.m.functions:
        for blk in f.blocks:
            out, changed = [], False
            for ins in blk.instructions:
                si = ins.sync_info
                if (si is not None and len(si.on_wait) > 1
                        and type(ins).__name__ not in skip):
                    waits = list(si.on_wait)
                    for i, w in enumerate(waits[:-1]):
                        out.append(mybir.InstNoOp(
                            name=f"{ins.name}-w{i}",
                            engine=ins.engine,
                            sync_info=mybir.SyncInfo(on_wait=[w], on_update=[]),
                            bass_nofuse=True,
                        ))
                    ins.sync_info = mybir.SyncInfo(
                        on_wait=[waits[-1]], on_update=list(si.on_update))
                    changed = True
                out.append(ins)
            if changed:
                blk.instructions = out


def _get_program(T: int, masked: bool):
    key = (T, masked)
    if key not in _CACHE:
        _CACHE[key] = _build(T, masked=masked)
    return _CACHE[key]


def _prep_core(seq_c, hid_c, mask_c, W_seq, W_h, v_att, W_ih, W_hh, b_ih, b_hh,
               masked=False):
    """Host-side layout prep for one core's batch slice."""
    bf16 = ml_dtypes.bfloat16
    f32 = np.float32
    # seqT [128, BL, 2, N]: seqT[r, b, dh, n] = seq[b, n, dh*128+r]
    seqT = np.ascontiguousarray(
        seq_c.transpose(2, 0, 1).reshape(2, 128, BL, N).transpose(1, 2, 0, 3)
    ).astype(bf16)
    # seqO fp8 DoubleRow layout [128, BL, N/256, 2, D]:
    #   seqO[r, b, c, kk, d] = seq[b, c*256 + kk*128 + r, d]
    seqO = np.ascontiguousarray(
        seq_c.reshape(BL, N // 256, 2, 128, D).transpose(3, 0, 1, 2, 4)
    ).astype(ml_dtypes.float8_e4m3)
    wseq = np.ascontiguousarray(
        W_seq.reshape(2, 128, P).transpose(1, 0, 2)
    ).astype(bf16)
    v8 = np.zeros((128, 2, BL, BL), dtype=f32)
    vr = v_att.reshape(2, 128).transpose(1, 0)  # [128, 2]
    for b in range(BL):
        v8[:, :, b, b] = vr
    v8 = v8.astype(bf16)
    h0T = np.ascontiguousarray(
        hid_c.transpose(1, 0).reshape(2, 128, BL).transpose(1, 0, 2)
    ).astype(f32)
    wh = np.ascontiguousarray(
        W_h.reshape(2, 128, P).transpose(1, 0, 2)
    ).astype(bf16)
    wihT = np.ascontiguousarray(
        W_ih.transpose(1, 0).reshape(2, 128, 3 * H).transpose(1, 0, 2)
    ).astype(bf16)
    whhT = np.ascontiguousarray(
        W_hh.transpose(1, 0).reshape(2, 128, 3 * H).transpose(1, 0, 2)
    ).astype(bf16)
    bih8 = np.broadcast_to(
        b_ih.reshape(6, 128).transpose(1, 0)[:, :, None], (128, 6, BL)
    ).astype(f32)
    bhh8 = np.broadcast_to(
        b_hh.reshape(6, 128).transpose(1, 0)[:, :, None], (128, 6, BL)
    ).astype(f32)
    out = {
        "seqT": seqT, "seqO": seqO, "wseq": wseq, "v8": v8,
        "id8": np.eye(BL, dtype=bf16), "id8f": np.eye(BL, dtype=f32),
        "h0T": h0T, "wh": wh, "wihT": wihT, "whhT": whhT,
        "bih8": np.ascontiguousarray(bih8), "bhh8": np.ascontiguousarray(bhh8),
    }
    if masked:
        out["mb8"] = np.where(mask_c > 0, 0.0, NEG_INF).astype(bf16)
    return out


def kernel(sequence, hidden_t, sequence_mask, num_steps,
           W_seq, W_h, v_att, W_ih, W_hh, b_ih, b_hh):
    from concourse.bass_utils import run_bass_kernel_spmd

    T = int(num_steps)
    sequence = np.asarray(sequence, np.float32)
    hidden_t = np.asarray(hidden_t, np.float32)
    sequence_mask = np.asarray(sequence_mask, np.float32)
    W_seq = np.asarray(W_seq, np.float32)
    W_h = np.asarray(W_h, np.float32)
    v_att = np.asarray(v_att, np.float32)
    W_ih = np.asarray(W_ih, np.float32)
    W_hh = np.asarray(W_hh, np.float32)
    b_ih = np.asarray(b_ih, np.float32)
    b_hh = np.asarray(b_hh, np.float32)

    masked = bool(np.any(sequence_mask <= 0))
    nc = _get_program(T, masked)
    in_maps = []
    for c in range(NCORES):
        sl = slice(c * BL, (c + 1) * BL)
        in_maps.append(_prep_core(
            sequence[sl], hidden_t[sl], sequence_mask[sl],
            W_seq, W_h, v_att, W_ih, W_hh, b_ih, b_hh, masked=masked,
        ))
    kr = run_bass_kernel_spmd(
        nc, in_maps, list(range(NCORES)), **_RUN_KWARGS,
    )
    globals()["_LAST_RESULTS"] = kr
    res = kr.results
    outs = []
    for c in range(NCORES):
        w_e = res[c]["wout"].astype(np.float32)        # [BL, T, N]
        es = res[c]["esum"].astype(np.float32)         # [T, BL]
        outs.append(w_e / es.transpose(1, 0)[:, :, None])
    return np.concatenate(outs, axis=0)


# test-harness hooks (unused in grading): set _RUN_KWARGS = {"trace": True}
# before calling kernel() to get NTFF profile info in _LAST_RESULTS.
_RUN_KWARGS = {}
_LAST_RESULTS = None


# revision 55
# speedup vs baseline: 1.0220x; 1.0220x over previous
"""Pointer-network decode kernel for 8 Trainium2 NeuronCores (v2).

Data-parallel over batch: B=64 -> 8 batches per core. Each core runs the
full T-step attention/GRU decode for its batch slice; the host shards
inputs (with layout pre-transposes/casts) and concatenates outputs.

Per core (B_loc=8, N=2048, D=H=P=256):
  t=0: proj = W_seq^T seqT (PE, bf16, psum groups [128, 1024]); tanh w/
       hW-bias (ACT, psum -> bf16 th); proj cache psum -> bf16 (DVE);
       score matmuls v8 x th accumulate into psc[8, 2048] on top of a
       mask-bias init pass (identity-stationary matmul).  Emission is
       software-pipelined: scores for group g-1 issue behind proj g.
  t>0: tanh from bf16 proj cache, same lagged score matmuls.
  softmax: exp straight from psum (no max subtraction; scores are O(1)),
       unnormalized bf16 w rows + fp32 row-sum accum; host divides.
  vec: transpose w rows (PE) -> wT bf16; vec_b = sum_c wT-col x seqO
       chunk matmuls; normalized against the exp row-sum via a
       reciprocal row broadcast (gpsimd) folded into the psum evac.
  GRU in transposed space: PE bf16 matmuls, psum evacuated fused with
       bias add (DVE scalar_tensor_tensor), fp32 gate math.
All PE stationaries are bf16 (fp32 stationaries cost a double LDWEIGHTS
pass).  Host normalizes wout by esum and upcasts to fp32.
"""

import numpy as np
import ml_dtypes

B, N, D, H, P = 64, 2048, 256, 256, 256
NCORES = 8
BL = B // NCORES          # batch per core
NEG_INF = -1e9

_CACHE = {}


def _build(T: int, split: bool = True, masked: bool = False):
    """Build the Bass program (one core's SPMD program) for T decode steps.

    masked=False specializes for an all-ones sequence_mask (the mask bias
    rows are all zero, so the psc init pass is dropped and the first score
    matmul starts the accumulation)."""
    from contextlib import ExitStack
    import concourse.bass as bass
    import concourse.tile as tile
    from concourse import mybir, library_config

    f32 = mybir.dt.float32
    bf16 = mybir.dt.bfloat16
    f8 = mybir.dt.float8e4
    AF = mybir.ActivationFunctionType
    ALU = mybir.AluOpType
    DR = mybir.MatmulPerfMode.DoubleRow

    nc = bass.Bass()

    # ---- DRAM I/O (per-core shapes) ----
    d_seqT = nc.dram_tensor("seqT", [128, BL, 2, N], bf16, kind="ExternalInput")
    d_seqO = nc.dram_tensor("seqO", [128, BL, N // 256, 2, D], f8,
                            kind="ExternalInput")
    d_wseq = nc.dram_tensor("wseq", [128, 2, P], bf16, kind="ExternalInput")
    d_v8 = nc.dram_tensor("v8", [128, 2, BL, BL], bf16, kind="ExternalInput")
    if masked:
        d_mb8 = nc.dram_tensor("mb8", [BL, N], bf16, kind="ExternalInput")
    d_id8 = nc.dram_tensor("id8", [BL, BL], bf16, kind="ExternalInput")
    d_id8f = nc.dram_tensor("id8f", [BL, BL], f32, kind="ExternalInput")
    d_h0T = nc.dram_tensor("h0T", [128, 2, BL], f32, kind="ExternalInput")
    d_wh = nc.dram_tensor("wh", [128, 2, P], bf16, kind="ExternalInput")
    d_wihT = nc.dram_tensor("wihT", [128, 2, 3 * H], bf16, kind="ExternalInput")
    d_whhT = nc.dram_tensor("whhT", [128, 2, 3 * H], bf16, kind="ExternalInput")
    d_bih8 = nc.dram_tensor("bih8", [128, 6, BL], f32, kind="ExternalInput")
    d_bhh8 = nc.dram_tensor("bhh8", [128, 6, BL], f32, kind="ExternalInput")
    d_wout = nc.dram_tensor("wout", [BL, T, N], bf16, kind="ExternalOutput")
    d_esum = nc.dram_tensor("esum", [T, BL], f32, kind="ExternalOutput")

    NC4 = N // 512    # score psum banks
    NCH = N // 128    # seqO chunks

    with tile.TileContext(nc) as tc, ExitStack() as ctx:
        # PSUM (16KB/partition): psc [8, 2048] f32 8K + pg 2 x 4K.  All
        # small matmul outputs borrow pg slots outside the proj pipeline.
        pscp = ctx.enter_context(tc.tile_pool(name="psc", bufs=1, space="PSUM"))
        pgp = ctx.enter_context(tc.tile_pool(name="pg", bufs=2, space="PSUM"))

        cpool = ctx.enter_context(tc.tile_pool(name="consts", bufs=1))
        blkp = ctx.enter_context(tc.tile_pool(name="blk", bufs=13))
        sqop = ctx.enter_context(tc.tile_pool(name="sqo", bufs=8))
        thp = ctx.enter_context(tc.tile_pool(name="th", bufs=6))
        rowp = ctx.enter_context(tc.tile_pool(name="rows", bufs=2))
        hp = ctx.enter_context(tc.tile_pool(name="hstate", bufs=2))

        # ---- constants / weights into SBUF (step-0-critical first) ----
        wseq_sb = cpool.tile([128, 2, P], bf16, tag="wseq")
        nc.sync.dma_start(wseq_sb[:], d_wseq[:])
        hT0 = hp.tile([128, 2, BL], f32, tag="hT")
        nc.sync.dma_start(hT0[:], d_h0T[:])
        wh_sb = cpool.tile([128, 2, P], bf16, tag="wh")
        nc.sync.dma_start(wh_sb[:], d_wh[:])
        ones_f = cpool.tile([1, 128], f32, tag="ones_f")
        nc.gpsimd.memset(ones_f[:], 1.0)

        # ---- sequence loads: front batches split finer so b0 lands fast;
        # n-major piece order so the first proj group waits on 4 pieces ----
        seqT_b = [blkp.tile([128, 2, N], bf16, tag="blk", name=f"seqT{b}")
                  for b in range(BL)]

        def seq_pieces(b):
            tl = seqT_b[b]
            nsplit = 4 if b < 2 else 2
            for q in range(nsplit):
                qs = slice(q * (N // nsplit), (q + 1) * (N // nsplit))
                for dh in range(2):
                    nc.sync.dma_start(tl[:, dh, qs], d_seqT[:, b, dh, qs])

        seq_pieces(0)
        # scores (v8) and the transposes (id8) are not needed until a few
        # microseconds in; b0's pieces go out first
        v8_sb = cpool.tile([128, 2, BL, BL], bf16, tag="v8")
        nc.sync.dma_start(v8_sb[:], d_v8[:])
        if masked:
            mb8_sb = cpool.tile([BL, N], bf16, tag="mb8")
            nc.sync.dma_start(mb8_sb[:], d_mb8[:])
        seq_pieces(1)
        id8_sb = cpool.tile([BL, BL], bf16, tag="id8")
        nc.sync.dma_start(id8_sb[:], d_id8[:])
        id8f_sb = cpool.tile([BL, BL], f32, tag="id8f")
        nc.sync.dma_start(id8f_sb[:], d_id8f[:])
        for b in range(2, BL):
            seq_pieces(b)

        wihT_sb = cpool.tile([128, 2, 3 * H], bf16, tag="wihT")
        nc.sync.dma_start(wihT_sb[:], d_wihT[:])
        whhT_sb = cpool.tile([128, 2, 3 * H], bf16, tag="whhT")
        nc.sync.dma_start(whhT_sb[:], d_whhT[:])
        bih8_sb = cpool.tile([128, 6, BL], f32, tag="bih8")
        nc.sync.dma_start(bih8_sb[:], d_bih8[:])
        bhh8_sb = cpool.tile([128, 6, BL], f32, tag="bhh8")
        nc.sync.dma_start(bhh8_sb[:], d_bhh8[:])

        seqO_b = []
        for b in range(BL):
            tl = sqop.tile([128, NCH // 2, 2, D], f8, tag="sqo",
                           name=f"seqO{b}")
            nc.sync.dma_start(tl[:], d_seqO[:, b, :, :, :])
            seqO_b.append(tl)

        projc_b = [None] * BL
        hT_sb = hT0
        hTb_sb = hp.tile([128, 2, BL], bf16, tag="hTb")
        nc.vector.tensor_copy(hTb_sb[:], hT_sb[:])

        def compute_hW(hW_sb, hTb, bs, bw, tag):
            """hW^T = W_h^T h for batches [bs, bs+bw) -> hW_sb[:, :, bs:]."""
            p_hw = pgp.tile([128, 2, bw], f32, tag="pg", name=f"p_hw{tag}")
            for mh in range(2):
                for kh in range(2):
                    nc.tensor.matmul(
                        p_hw[:, mh, :],
                        wh_sb[:, kh, mh * 128:(mh + 1) * 128],
                        hTb[:, kh, bs:bs + bw],
                        start=(kh == 0), stop=(kh == 1),
                    )
            nc.scalar.copy(hW_sb[:, :, bs:bs + bw], p_hw[:])

        hW_sb = hp.tile([128, 2, BL], f32, tag="hW")
        compute_hW(hW_sb, hTb_sb, 0, BL, "0")

        for t in range(T):
            # ---- score psum (mask-bias init only when the mask is real) ----
            psc = pscp.tile([BL, N], f32, tag="sc", name=f"psc{t}")
            started = set()
            if masked:
                for c in range(NC4):
                    cs = slice(c * 512, (c + 1) * 512)
                    nc.tensor.matmul(
                        psc[:, cs], id8_sb[:], mb8_sb[:, cs],
                        start=True, stop=False, skip_group_check=True,
                    )
                    started.add(c)

            # ---- proj(+cache)/tanh, with score matmuls lagged one group ----
            def emit_score(g):
                # each psc bank's final accumulation lands in a (b7, mh1)
                # group (banks are written by disjoint ch groups at t=0)
                b, mh, th_t, ns = g
                for c2 in range(len(ns)):
                    c = ns[c2] // 512
                    nc.tensor.matmul(
                        psc[:, ns[c2]:ns[c2] + 512],
                        v8_sb[:, mh, b, :],
                        th_t[:, c2 * 512:(c2 + 1) * 512],
                        start=(c not in started),
                        stop=(b == BL - 1 and mh == 1),
                        skip_group_check=True,
                    )
                    started.add(c)

            pend = []
            for b in range(BL):
                for mh in range(2):
                    if t == 0:
                        if projc_b[b] is None:
                            projc_b[b] = blkp.tile(
                                [128, 2, N], bf16, tag="blk", name=f"projc{b}")
                        for ch in range(2):
                            cs = slice(ch * 1024, (ch + 1) * 1024)
                            pg = pgp.tile([128, 1024], f32, tag="pg",
                                          name=f"pg{b}_{mh}_{ch}")
                            for kh in range(2):
                                for c2 in range(2):
                                    ns = slice(ch * 1024 + c2 * 512,
                                               ch * 1024 + (c2 + 1) * 512)
                                    nc.tensor.matmul(
                                        pg[:, c2 * 512:(c2 + 1) * 512],
                                        wseq_sb[:, kh, mh * 128:(mh + 1) * 128],
                                        seqT_b[b][:, kh, ns],
                                        start=(kh == 0), stop=(kh == 1),
                                    )
                                # score matmuls (lagged 2 groups) slot
                                # between the kh passes, widening the psum
                                # accumulation RAW distance
                                if kh == 0 and len(pend) >= 2:
                                    emit_score(pend.pop(0))
                            th = thp.tile([128, 1024], bf16, tag="th")
                            nc.scalar.activation(
                                th[:], pg[:], AF.Tanh,
                                bias=hW_sb[:, mh, b:b + 1],
                            )
                            if T > 1:
                                nc.vector.tensor_copy(projc_b[b][:, mh, cs],
                                                      pg[:])
                            pend.append((b, mh, th,
                                         [ch * 1024, ch * 1024 + 512]))
                    else:
                        if len(pend) >= 2:
                            emit_score(pend.pop(0))
                        th1 = thp.tile([128, N], bf16, tag="th")
                        nc.scalar.activation(
                            th1[:], projc_b[b][:, mh, :], AF.Tanh,
                            bias=hW_sb[:, mh, b:b + 1],
                        )
                        pend.append((b, mh, th1, [0, 512, 1024, 1536]))
            while pend:
                emit_score(pend.pop(0))

            HB = BL // 2

            def gates_T(src_bf, wT_w, bias8, bs, name):
                pgt = pgp.tile([128, 6, HB], f32, tag="pg", name=f"pgt{name}")
                for gh in range(6):
                    for dh in range(2):
                        nc.tensor.matmul(
                            pgt[:, gh, :],
                            wT_w[:, dh, gh * 128:(gh + 1) * 128],
                            src_bf[:, dh, :],
                            start=(dh == 0), stop=(dh == 1),
                        )
                g_sb = hp.tile([128, 6, HB], f32, tag=f"g{name[-1]}")
                nc.vector.scalar_tensor_tensor(
                    g_sb[:], pgt[:], 1.0, bias8[:, :, bs],
                    op0=ALU.bypass, op1=ALU.add,
                )
                return g_sb

            # h-gates don't depend on this step's attention: emit them
            # before exp so the PE computes them during the softmax
            gh_halves = []
            if t < T - 1:
                for half in range(2):
                    bs = slice(half * HB, half * HB + HB)
                    gh_halves.append(gates_T(
                        hTb_sb[:, :, bs], whhT_sb, bhh8_sb, bs,
                        f"gh{t}_{half}"))

            # ---- softmax numerator: exp straight from psum, bf16 out ----
            w_eb = rowp.tile([BL, N], bf16, tag="w_eb")
            esum = rowp.tile([BL, 1], f32, tag="esum")
            nc.scalar.activation(
                w_eb[:], psc[:], AF.Exp, accum_out=esum[:],
            )
            nc.sync.dma_start(d_wout[:, t, :], w_eb[:])
            nc.sync.dma_start(d_esum[t, :], esum[:, 0])

            if t == T - 1:
                break   # final step: w/esum are out, the GRU state is dead

            # ---- wT: transpose w rows into [n-part, b] fp8 chunks ----
            # (emitted first: they only need w_eb, so the PE is not queued
            # behind the esum-transpose/reciprocal chain)
            wT_sb = rowp.tile([128, NCH, 2 * BL], f8, tag="wT")
            nc.gpsimd.memset(wT_sb[:, :, 1:2 * BL:2], 0.0)
            p_wT = pgp.tile([128, NCH, BL], bf16, tag="pg", name=f"p_wT{t}")
            for c in range(NCH):
                nc.tensor.transpose(
                    p_wT[:, c, :], w_eb[:, c * 128:(c + 1) * 128], id8_sb[:],
                )
            # ---- 1/esum as a broadcast row set [128, 8] fp32 ----
            p_esT = pgp.tile([1, BL], f32, tag="pg", name=f"p_esT{t}")
            nc.tensor.transpose(p_esT[:], esum[:], id8f_sb[:])
            rrow = rowp.tile([1, BL], f32, tag="rrow")
            nc.vector.reciprocal(rrow[:], p_esT[:])
            p_rb = pgp.tile([128, BL], f32, tag="pg", name=f"p_rb{t}")
            nc.tensor.matmul(p_rb[:], ones_f[:], rrow[:], start=True, stop=True)
            rb = rowp.tile([128, BL], f32, tag="rb")
            nc.vector.tensor_copy(rb[:], p_rb[:])
            # dual-fp8 ldweights needs a 16-bit-aligned base and k-stride
            # and >1 column: batch b's weights live at even column 2b with
            # a zero pad at 2b+1 (its psum row is never read).
            nc.vector.tensor_copy(wT_sb[:, :, 0:2 * BL:2], p_wT[:])

            # ---- vec + GRU + next hW, in batch halves: the second half's
            # work overlaps the next step's tanh for the first half ----
            p_vT = pscp.tile([128, 2, BL], f32, tag="sc", name=f"p_vT{t}")
            hT_new = hp.tile([128, 2, BL], f32, tag="hT", name=f"hT{t}")
            hTb_new = hp.tile([128, 2, BL], bf16, tag="hTb", name=f"hTb{t}")
            hW_new = hp.tile([128, 2, BL], f32, tag="hW", name=f"hW{t}")
            HB = BL // 2
            for half in range(2):
                bs = slice(half * HB, half * HB + HB)
                # vec rows via fp8 DoubleRow (256-wide contraction/pass; vec
                # feeds only the GRU whose output sensitivity is ~2%, so
                # fp8 noise is invisible)
                for b in range(half * HB, half * HB + HB):
                    p_vr = pgp.tile([2, D], f32, tag="pg",
                                    name=f"p_vr{t}_{b}")
                    for c in range(NCH // 2):
                        nc.tensor.matmul(
                            p_vr[:], wT_sb[:, 2 * c:2 * c + 2,
                                           2 * b:2 * b + 2],
                            seqO_b[b][:, c, :, :],
                            start=(c == 0), stop=(c == NCH // 2 - 1),
                            skip_group_check=True, perf_mode=DR,
                        )
                    vrow = rowp.tile([1, D], f32, tag="vrow")
                    nc.scalar.copy(vrow[:], p_vr[0:1, :])
                    for dh in range(2):
                        nc.tensor.transpose(
                            p_vT[:, dh, b:b + 1],
                            vrow[0:1, dh * 128:(dh + 1) * 128],
                            id8f_sb[:1, :1],
                        )
                # normalize + cast: vecTb = (vec_e^T) * (1/esum) in bf16
                vecTb = hp.tile([128, 2, HB], bf16, tag="vecTb")
                for dh in range(2):
                    nc.vector.tensor_tensor(
                        vecTb[:, dh, :], p_vT[:, dh, bs], rb[:, bs],
                        op=ALU.mult)

                gx = gates_T(vecTb, wihT_sb, bih8_sb, bs, f"gx{t}_{half}")
                gh_ = gh_halves[half]

                # r,z = sigmoid via 0.5*tanh(0.5x)+0.5
                rz_in = hp.tile([128, 4, HB], f32, tag="rz_in")
                nc.vector.tensor_tensor(rz_in[:], gx[:, 0:4, :],
                                        gh_[:, 0:4, :], op=ALU.add)
                rz = hp.tile([128, 4, HB], f32, tag="rz")
                nc.scalar.activation(rz[:], rz_in[:], AF.Tanh, scale=0.5)
                nc.vector.tensor_scalar(
                    rz[:], rz[:], 0.5, 0.5, op0=ALU.mult, op1=ALU.add,
                )
                # n = tanh(xn + r*hn)
                n_in = hp.tile([128, 2, HB], f32, tag="n_in")
                nc.vector.tensor_tensor(n_in[:], rz[:, 0:2, :],
                                        gh_[:, 4:6, :], op=ALU.mult)
                nc.vector.tensor_tensor(n_in[:], n_in[:], gx[:, 4:6, :],
                                        op=ALU.add)
                n_t = hp.tile([128, 2, HB], f32, tag="n_t")
                nc.scalar.activation(n_t[:], n_in[:], AF.Tanh)
                # h' = n + z*(h - n)
                hmn = hp.tile([128, 2, HB], f32, tag="hmn")
                nc.vector.tensor_tensor(hmn[:], hT_sb[:, :, bs], n_t[:],
                                        op=ALU.subtract)
                nc.vector.tensor_tensor(hmn[:], rz[:, 2:4, :], hmn[:],
                                        op=ALU.mult)
                nc.vector.tensor_tensor(hT_new[:, :, bs], n_t[:], hmn[:],
                                        op=ALU.add)
                nc.vector.tensor_copy(hTb_new[:, :, bs], hT_new[:, :, bs])
                compute_hW(hW_new, hTb_new, half * HB, HB, f"{t}_{half}")
            hT_sb = hT_new
            hTb_sb = hTb_new
            hW_sb = hW_new

    if split:
        _split_multiwaits(nc, mybir)
    return nc


def _split_multiwaits(nc, mybir):
    """Walrus gives each lowered TPB instruction a single sem-wait slot;
    Tile happily emits several. Peel surplus waits onto same-engine NoOps
    inserted right before the instruction (semantically identical: the
    engine stalls at the same program point)."""
    skip = ("InstNoOp", "InstEventSemaphore")
    for f in nc.m.functions:
        for blk in f.blocks:
            out, changed = [], False
            for ins in blk.instructions:
                si = ins.sync_info
                if (si is not None and len(si.on_wait) > 1
                        and type(ins).__name__ not in skip):
                    waits = list(si.on_wait)
                    for i, w in enumerate(waits[:-1]):
                        out.append(mybir.InstNoOp(
                            name=f"{ins.name}-w{i}",
                            engine=ins.engine,
                            sync_info=mybir.SyncInfo(on_wait=[w], on_update=[]),
                            bass_nofuse=True,
                        ))
                    ins.sync_info = mybir.SyncInfo(
                        on_wait=[waits[-1]], on_update=list(si.on_update))
                    changed = True
                out.append(ins)
            if changed:
                blk.instructions = out


def _get_program(T: int, masked: bool):
    key = (T, masked)
    if key not in _CACHE:
        _CACHE[key] = _build(T, masked=masked)
    return _CACHE[key]


def _prep_core(seq_c, hid_c, mask_c, W_seq, W_h, v_att, W_ih, W_hh, b_ih, b_hh,
               masked=False):
    """Host-side layout prep for one core's batch slice."""
    bf16 = ml_dtypes.bfloat16
    f32 = np.float32
    # seqT [128, BL, 2, N]: seqT[r, b, dh, n] = seq[b, n, dh*128+r]
    seqT = np.ascontiguousarray(
        seq_c.transpose(2, 0, 1).reshape(2, 128, BL, N).transpose(1, 2, 0, 3)
    ).astype(bf16)
    # seqO fp8 DoubleRow layout [128, BL, N/256, 2, D]:
    #   seqO[r, b, c, kk, d] = seq[b, c*256 + kk*128 + r, d]
    seqO = np.ascontiguousarray(
        seq_c.reshape(BL, N // 256, 2, 128, D).transpose(3, 0, 1, 2, 4)
    ).astype(ml_dtypes.float8_e4m3)
    wseq = np.ascontiguousarray(
        W_seq.reshape(2, 128, P).transpose(1, 0, 2)
    ).astype(bf16)
    v8 = np.zeros((128, 2, BL, BL), dtype=f32)
    vr = v_att.reshape(2, 128).transpose(1, 0)  # [128, 2]
    for b in range(BL):
        v8[:, :, b, b] = vr
    v8 = v8.astype(bf16)
    h0T = np.ascontiguousarray(
        hid_c.transpose(1, 0).reshape(2, 128, BL).transpose(1, 0, 2)
    ).astype(f32)
    wh = np.ascontiguousarray(
        W_h.reshape(2, 128, P).transpose(1, 0, 2)
    ).astype(bf16)
    wihT = np.ascontiguousarray(
        W_ih.transpose(1, 0).reshape(2, 128, 3 * H).transpose(1, 0, 2)
    ).astype(bf16)
    whhT = np.ascontiguousarray(
        W_hh.transpose(1, 0).reshape(2, 128, 3 * H).transpose(1, 0, 2)
    ).astype(bf16)
    bih8 = np.broadcast_to(
        b_ih.reshape(6, 128).transpose(1, 0)[:, :, None], (128, 6, BL)
    ).astype(f32)
    bhh8 = np.broadcast_to(
        b_hh.reshape(6, 128).transpose(1, 0)[:, :, None], (128, 6, BL)
    ).astype(f32)
    out = {
        "seqT": seqT, "seqO": seqO, "wseq": wseq, "v8": v8,
        "id8": np.eye(BL, dtype=bf16), "id8f": np.eye(BL, dtype=f32),
        "h0T": h0T, "wh": wh, "wihT": wihT, "whhT": whhT,
        "bih8": np.ascontiguousarray(bih8), "bhh8": np.ascontiguousarray(bhh8),
    }
    if masked:
        out["mb8"] = np.where(mask_c > 0, 0.0, NEG_INF).astype(bf16)
    return out


def kernel(sequence, hidden_t, sequence_mask, num_steps,
           W_seq, W_h, v_att, W_ih, W_hh, b_ih, b_hh):
    from concourse.bass_utils import run_bass_kernel_spmd

    T = int(num_steps)
    sequence = np.asarray(sequence, np.float32)
    hidden_t = np.asarray(hidden_t, np.float32)
    sequence_mask = np.asarray(sequence_mask, np.float32)
    W_seq = np.asarray(W_seq, np.float32)
    W_h = np.asarray(W_h, np.float32)
    v_att = np.asarray(v_att, np.float32)
    W_ih = np.asarray(W_ih, np.float32)
    W_hh = np.asarray(W_hh, np.float32)
    b_ih = np.asarray(b_ih, np.float32)
    b_hh = np.asarray(b_hh, np.float32)

    masked = bool(np.any(sequence_mask <= 0))
    nc = _get_program(T, masked)
    in_maps = []
    for c in range(NCORES):
        sl = slice(c * BL, (c + 1) * BL)
        in_maps.append(_prep_core(
            sequence[sl], hidden_t[sl], sequence_mask[sl],
            W_seq, W_h, v_att, W_ih, W_hh, b_ih, b_hh, masked=masked,
        ))
    kr = run_bass_kernel_spmd(
        nc, in_maps, list(range(NCORES)), **_RUN_KWARGS,
    )
    globals()["_LAST_RESULTS"] = kr
    res = kr.results
    outs = []
    for c in range(NCORES):
        w_e = res[c]["wout"].astype(np.float32)        # [BL, T, N]
        es = res[c]["esum"].astype(np.float32)         # [T, BL]
        outs.append(w_e / es.transpose(1, 0)[:, :, None])
    return np.concatenate(outs, axis=0)


# test-harness hooks (unused in grading): set _RUN_KWARGS = {"trace": True}
# before calling kernel() to get NTFF profile info in _LAST_RESULTS.
_RUN_KWARGS = {}
_LAST_RESULTS = None
